# revision 16
# baseline (speedup 1.0000x reference)
"""DiffMamba cross-attention kernel for 8 Trainium2 NeuronCores.

Problem (hardcoded shapes): B=4, SQ=SK=2048, D=1024, H=16, HD=64.
  q = x @ Wq.T ; k = e @ Wk.T ; v = e @ Wv.T      (per-head split, HD=64)
  out = softmax(q k^T / 8) v                       (merged heads)

Sharding: core c -> (batch b = c//2, head-group hg = c%2).  Each core owns
one batch element and 8 of the 16 heads (rows hg*512:(hg+1)*512 of W), so
all cores are fully independent (no collectives).

Host pre-transposes everything so the device kernel is transpose-free:
  xT [1024,2048], eT [1024,2048], wqT/wkT/wvT [1024,512]  (wqT pre-scaled 1/8)
Device computes outT [512,2048] = (attention output).T; host transposes back.

Device dataflow (bf16 matmul operands, fp32 PSUM accumulate -> full PE rate):
  phase 1: qT = wqT.T @ xT   [512,2048]   (head-dim on partitions)
           kT = wkT.T @ eT   [512,2048]
           v  = (eT.T @ wvT) [2048,512]   stored per-SK-tile augmented with a
                ones column per head ([128,16,8,65]) so the ctx matmul also
                produces the softmax denominator (row 64).
  phase 2: per head-pair p, per 512-wide query chunk c:
           for each of 16 SK tiles j:
             ST[j*128.., 0:512]   = kT_A.T @ qT_A   (rows 0-63 of PE)
             ST[j*128.., 512:1024]= kT_B.T @ qT_B   (rows 64-127, concurrent)
             PT = exp(ST)                            (one wide ACT op)
             ctx_A[65,512] += v_aug_A.T @ PT_A       (PSUM accumulate)
             ctx_B[65,512] += v_aug_B.T @ PT_B
           recip of denom rows (DVE), broadcast via K=2 selector matmul,
           normalize with DVE multiplies, DMA out.
"""

import os
import sys

import numpy as np

_REPO = "/opt/trn_rl_repo"
if os.path.isdir(_REPO) and _REPO not in sys.path:
    sys.path.insert(0, _REPO)

import concourse.bass as bass
import concourse.tile as tile
from concourse import bacc
from concourse import mybir
from concourse.bass_utils import run_bass_kernel_spmd

F32 = mybir.dt.float32
F32R = mybir.dt.float32r
BF16 = mybir.dt.bfloat16
PSUM = bass.MemorySpace.PSUM
EXP = mybir.ActivationFunctionType.Exp

B, S, D = 4, 2048, 1024
DL = 512          # head dims per core (8 heads x 64)
HL = 8            # local heads
NP = 4            # local head pairs
KT = D // 128     # 8 contraction tiles
NCORES = 8

_CACHE = {}
LAST_RESULT = None  # BassKernelResults of the most recent run (for profiling)


_DEBUG = os.environ.get("KBG_DEBUG") == "1"


def _build_program():
    # Bacc (not raw Bass): its compile pipeline splits multi-sem waits into
    # EventSemaphore instructions and moves matmul waits onto ldweights --
    # walrus rejects >1 sync wait on most instructions.
    nc = bacc.Bacc()
    dbg = {}
    if _DEBUG:
        dbg["qt"] = nc.declare_dram_parameter("dbg_qt", [128, S], F32, isOutput=True)
        dbg["kt"] = nc.declare_dram_parameter("dbg_kt", [128, S], F32, isOutput=True)
        dbg["va"] = nc.declare_dram_parameter("dbg_va", [128, 520], F32, isOutput=True)
        dbg["st"] = nc.declare_dram_parameter("dbg_st", [128, 1024], F32, isOutput=True)
        dbg["pt"] = nc.declare_dram_parameter("dbg_pt", [128, 1024], F32, isOutput=True)
        dbg["ctx"] = nc.declare_dram_parameter("dbg_ctx", [65, 512], F32, isOutput=True)
        dbg["dn"] = nc.declare_dram_parameter("dbg_dn", [1, 1024], F32, isOutput=True)
        dbg["rc"] = nc.declare_dram_parameter("dbg_rc", [1, 1024], F32, isOutput=True)
        dbg["bcs"] = nc.declare_dram_parameter("dbg_bcs", [64, 512], F32, isOutput=True)
    xT_h = nc.declare_dram_parameter("xT", [D, S], BF16, isOutput=False)
    eT_h = nc.declare_dram_parameter("eT", [D, S], BF16, isOutput=False)
    wqT_h = nc.declare_dram_parameter("wqT", [D, DL], BF16, isOutput=False)
    wkT_h = nc.declare_dram_parameter("wkT", [D, DL], BF16, isOutput=False)
    wvT_h = nc.declare_dram_parameter("wvT", [D, DL], BF16, isOutput=False)
    outT_h = nc.declare_dram_parameter("outT", [DL, S], F32, isOutput=True)

    # [D, N] viewed as [128, KT, N]: partition p, ktile k -> row k*128+p
    xT_v = xT_h[:].rearrange("(k p) n -> p k n", p=128)
    eT_v = eT_h[:].rearrange("(k p) n -> p k n", p=128)
    wqT_v = wqT_h[:].rearrange("(k p) n -> p k n", p=128)
    wkT_v = wkT_h[:].rearrange("(k p) n -> p k n", p=128)
    wvT_v = wvT_h[:].rearrange("(k p) n -> p k n", p=128)

    with tile.TileContext(nc) as tc:
        with tc.tile_pool(name="persist", bufs=1) as persist:
            qT = persist.tile([128, NP, S], BF16, tag="qT")
            kT = persist.tile([128, NP, S], BF16, tag="kT")
            # v augmented: per SK tile, per head: 64 v-dims + ones column
            vA = persist.tile([128, 16, HL, 65], BF16, tag="vA")
            # ones row living at partition 64 (the partition where the ctx
            # matmul's denominator row lands); lhsT for the broadcast matmuls
            ones_t = persist.tile([65, 128], F32, tag="ones_t")
            zbias = persist.tile([128, 1], F32, tag="zbias")

            nc.vector.memset(zbias[:], 0.0)
            nc.vector.memset(ones_t[64:65, :], 1.0)
            nc.vector.memset(vA[:, :, :, 64:65], 1.0)

            # ---------------- phase 1: projections ----------------
            with (
                tc.tile_pool(name="ph1", bufs=2) as ph1,
                tc.tile_pool(name="wp", bufs=1) as wp,
                tc.tile_pool(name="ps1", bufs=3, space=PSUM) as ps1,
            ):
                wq = wp.tile([128, KT, DL], BF16, tag="wq")
                wk = wp.tile([128, KT, DL], BF16, tag="wk")
                wv = wp.tile([128, KT, DL], BF16, tag="wv")
                nc.sync.dma_start(wq[:], wqT_v)
                nc.sync.dma_start(wk[:], wkT_v)
                nc.sync.dma_start(wv[:], wvT_v)

                # kT and v from encoder states; each eT chunk read once
                for n in range(4):
                    nsl = slice(n * 512, (n + 1) * 512)
                    et = ph1.tile([128, KT, 512], BF16, tag="act")
                    nc.sync.dma_start(et[:], eT_v[:, :, nsl])
                    for m in range(4):
                        ps = ps1.tile([128, 512], F32, tag="pp")
                        msl = slice(m * 128, (m + 1) * 128)
                        for k in range(KT):
                            nc.tensor.matmul(
                                ps[:],
                                wk[:, k, msl],
                                et[:, k, :],
                                start=(k == 0),
                                stop=(k == KT - 1),
                            )
                        nc.vector.tensor_copy(kT[:, m, nsl], ps[:])
                    for sub in range(4):
                        mj = n * 4 + sub
                        ps = ps1.tile([128, 512], F32, tag="pp")
                        ssl = slice(sub * 128, (sub + 1) * 128)
                        for k in range(KT):
                            nc.tensor.matmul(
                                ps[:],
                                et[:, k, ssl],
                                wv[:, k, :],
                                start=(k == 0),
                                stop=(k == KT - 1),
                            )
                        nc.vector.tensor_copy(
                            vA[:, mj, :, 0:64],
                            ps[:].rearrange("p (h d) -> p h d", h=HL),
                        )

                # qT from hidden states
                for n in range(4):
                    nsl = slice(n * 512, (n + 1) * 512)
                    xt = ph1.tile([128, KT, 512], BF16, tag="act")
                    nc.sync.dma_start(xt[:], xT_v[:, :, nsl])
                    for m in range(4):
                        ps = ps1.tile([128, 512], F32, tag="pp")
                        msl = slice(m * 128, (m + 1) * 128)
                        for k in range(KT):
                            nc.tensor.matmul(
                                ps[:],
                                wq[:, k, msl],
                                xt[:, k, :],
                                start=(k == 0),
                                stop=(k == KT - 1),
                            )
                        nc.vector.tensor_copy(qT[:, m, nsl], ps[:])

            # ---------------- phase 2: attention ----------------
            with (
                tc.tile_pool(name="stp", bufs=2, space=PSUM) as stp,
                tc.tile_pool(name="ctxp", bufs=2, space=PSUM) as ctxp,
                tc.tile_pool(name="bcp", bufs=2, space=PSUM) as bcp,
                tc.tile_pool(name="ptp", bufs=3) as ptp,
                tc.tile_pool(name="stg", bufs=2) as stgp,
                tc.tile_pool(name="dnp", bufs=2) as dnp,
                tc.tile_pool(name="dbgp", bufs=1) as dbgp,
            ):
                if _DEBUG:
                    t = dbgp.tile([128, S], F32, tag="dbg_qt")
                    nc.vector.tensor_copy(t[:], qT[:, 0, :])
                    nc.sync.dma_start(dbg["qt"][:], t[:])
                    t = dbgp.tile([128, S], F32, tag="dbg_kt")
                    nc.vector.tensor_copy(t[:], kT[:, 0, :])
                    nc.sync.dma_start(dbg["kt"][:], t[:])
                    t = dbgp.tile([128, 8, 65], F32, tag="dbg_va")
                    nc.vector.tensor_copy(t[:], vA[:, 0, :, :])
                    nc.sync.dma_start(
                        dbg["va"][:].rearrange("p (h d) -> p h d", h=HL), t[:]
                    )
                for p in range(NP):
                    stage_a = stgp.tile([64, S], F32, tag="stage_a")
                    stage_b = stgp.tile([64, S], F32, tag="stage_b")
                    for c in range(4):
                        csl = slice(c * 512, (c + 1) * 512)
                        ctx_a = ctxp.tile([65, 512], F32, tag="ctx")
                        ctx_b = ctxp.tile([65, 512], F32, tag="ctx")
                        for j in range(16):
                            jsl = slice(j * 128, (j + 1) * 128)
                            st = stp.tile([128, 1024], F32, tag="st")
                            nc.tensor.matmul(
                                st[:, 0:512],
                                kT[0:64, p, jsl],
                                qT[0:64, p, csl],
                                start=True,
                                stop=True,
                            )
                            nc.tensor.matmul(
                                st[:, 512:1024],
                                kT[64:128, p, jsl],
                                qT[64:128, p, csl],
                                start=True,
                                stop=True,
                            )
                            pt = ptp.tile([128, 1024], BF16, tag="pt")
                            nc.scalar.activation(pt[:], st[:], EXP, bias=zbias[:, 0:1])
                            nc.tensor.matmul(
                                ctx_a[:],
                                vA[:, j, 2 * p, :],
                                pt[:, 0:512],
                                start=(j == 0),
                                stop=(j == 15),
                            )
                            nc.tensor.matmul(
                                ctx_b[:],
                                vA[:, j, 2 * p + 1, :],
                                pt[:, 512:1024],
                                start=(j == 0),
                                stop=(j == 15),
                            )
                            if _DEBUG and p == 0 and c == 0 and j == 0:
                                t = dbgp.tile([128, 1024], F32, tag="dbg_st")
                                nc.vector.tensor_copy(t[:], st[:])
                                nc.sync.dma_start(dbg["st"][:], t[:])
                                t = dbgp.tile([128, 1024], F32, tag="dbg_pt")
                                nc.vector.tensor_copy(t[:], pt[:])
                                nc.sync.dma_start(dbg["pt"][:], t[:])
                        # denominators live in ctx row 64; keep everything at
                        # partition base 64 (32-aligned) until the broadcast
                        dn = dnp.tile([65, 1024], F32, tag="dn")
                        nc.vector.tensor_copy(dn[64:65, 0:512], ctx_a[64:65, :])
                        nc.vector.tensor_copy(dn[64:65, 512:1024], ctx_b[64:65, :])
                        rc = dnp.tile([65, 1024], F32, tag="rc")
                        nc.vector.reciprocal(rc[64:65, :], dn[64:65, :])
                        # broadcast 1/denom over 64 partitions via K=1 matmuls
                        bc_a = bcp.tile([64, 512], F32, tag="bc")
                        bc_b = bcp.tile([64, 512], F32, tag="bc")
                        nc.tensor.matmul(
                            bc_a[:],
                            ones_t[64:65, 0:64],
                            rc[64:65, 0:512],
                            start=True,
                            stop=True,
                        )
                        nc.tensor.matmul(
                            bc_b[:],
                            ones_t[64:65, 0:64],
                            rc[64:65, 512:1024],
                            start=True,
                            stop=True,
                        )
                        bcs_a = dnp.tile([64, 512], F32, tag="bcs_a")
                        bcs_b = dnp.tile([64, 512], F32, tag="bcs_b")
                        nc.vector.tensor_copy(bcs_a[:], bc_a[:])
                        nc.vector.tensor_copy(bcs_b[:], bc_b[:])
                        if _DEBUG and p == 0 and c == 0:
                            t = dbgp.tile([65, 512], F32, tag="dbg_ctx")
                            nc.vector.tensor_copy(t[:], ctx_a[:])
                            nc.sync.dma_start(dbg["ctx"][:], t[:])
                            nc.sync.dma_start(dbg["dn"][:], dn[64:65, :])
                            nc.sync.dma_start(dbg["rc"][:], rc[64:65, :])
                            nc.sync.dma_start(dbg["bcs"][:], bcs_a[:])
                        nc.vector.tensor_mul(
                            stage_a[:, csl], ctx_a[0:64, :], bcs_a[:]
                        )
                        nc.vector.tensor_mul(
                            stage_b[:, csl], ctx_b[0:64, :], bcs_b[:]
                        )
                    nc.sync.dma_start(
                        outT_h[p * 128 : p * 128 + 64, :], stage_a[:]
                    )
                    nc.sync.dma_start(
                        outT_h[p * 128 + 64 : (p + 1) * 128, :], stage_b[:]
                    )

    nc.finalize()
    return nc


def kernel(hidden_states, encoder_hidden_states, Wq, Wk, Wv):
    global LAST_RESULT
    hidden_states = np.asarray(hidden_states, dtype=np.float32)
    encoder_hidden_states = np.asarray(encoder_hidden_states, dtype=np.float32)
    Wq = np.asarray(Wq, dtype=np.float32)
    Wk = np.asarray(Wk, dtype=np.float32)
    Wv = np.asarray(Wv, dtype=np.float32)

    if "nc" not in _CACHE:
        _CACHE["nc"] = _build_program()
    nc = _CACHE["nc"]

    import ml_dtypes

    bf16 = ml_dtypes.bfloat16
    in_maps = []
    for c in range(NCORES):
        b, hg = divmod(c, 2)
        rsl = slice(hg * DL, (hg + 1) * DL)
        in_maps.append(
            {
                "xT": np.ascontiguousarray(hidden_states[b].T).astype(bf16),
                "eT": np.ascontiguousarray(encoder_hidden_states[b].T).astype(bf16),
                # fold the 1/sqrt(HD)=1/8 score scale into Wq
                "wqT": np.ascontiguousarray((Wq[rsl] * 0.125).T).astype(bf16),
                "wkT": np.ascontiguousarray(Wk[rsl].T).astype(bf16),
                "wvT": np.ascontiguousarray(Wv[rsl].T).astype(bf16),
            }
        )

    res = run_bass_kernel_spmd(nc, in_maps, list(range(NCORES)))
    LAST_RESULT = res

    out = np.empty((B, S, D), dtype=np.float32)
    for c in range(NCORES):
        b, hg = divmod(c, 2)
        out[b, :, hg * DL : (hg + 1) * DL] = res.results[c]["outT"].T
    return out


# revision 19
# speedup vs baseline: 1.4193x; 1.4193x over previous
"""DiffMamba cross-attention kernel for 8 Trainium2 NeuronCores.

Problem (hardcoded shapes): B=4, SQ=SK=2048, D=1024, H=16, HD=64.
  q = x @ Wq.T ; k = e @ Wk.T ; v = e @ Wv.T      (per-head split, HD=64)
  out = softmax(q k^T / 8) v                       (merged heads)

Sharding: core c -> (batch b = c//2, head-group hg = c%2).  Each core owns
one batch element and 8 of the 16 heads (rows hg*512:(hg+1)*512 of W), so
all cores are fully independent (no collectives).

Host pre-transposes everything so the device kernel is transpose-free:
  xT [1024,2048], eT [1024,2048], wqT/wkT/wvT [1024,512]  (wqT pre-scaled 1/8)
Device computes outT [512,2048] = (attention output).T; host transposes back.

Device dataflow (bf16 matmul operands, fp32 PSUM accumulate -> full PE rate):
  phase 1: qT = wqT.T @ xT   [512,2048]   (head-dim on partitions)
           kT = wkT.T @ eT   [512,2048]
           v  = (eT.T @ wvT) [2048,512]   stored per-SK-tile augmented with a
                ones column per head ([128,16,8,65]) so the ctx matmul also
                produces the softmax denominator (row 64).
  phase 2: per head-pair p, per 512-wide query chunk c:
           for each of 16 SK tiles j:
             ST[j*128.., 0:512]   = kT_A.T @ qT_A   (rows 0-63 of PE)
             ST[j*128.., 512:1024]= kT_B.T @ qT_B   (rows 64-127, concurrent)
             PT = exp(ST)                            (one wide ACT op)
             ctx_A[65,512] += v_aug_A.T @ PT_A       (PSUM accumulate)
             ctx_B[65,512] += v_aug_B.T @ PT_B
           recip of denom rows (DVE), broadcast via K=2 selector matmul,
           normalize with DVE multiplies, DMA out.
"""

import os
import sys

import numpy as np

_REPO = "/opt/trn_rl_repo"
if os.path.isdir(_REPO) and _REPO not in sys.path:
    sys.path.insert(0, _REPO)

import concourse.bass as bass
import concourse.tile as tile
from concourse import bacc
from concourse import mybir
from concourse.bass_utils import run_bass_kernel_spmd

F32 = mybir.dt.float32
F32R = mybir.dt.float32r
BF16 = mybir.dt.bfloat16
PSUM = bass.MemorySpace.PSUM
EXP = mybir.ActivationFunctionType.Exp

B, S, D = 4, 2048, 1024
DL = 512          # head dims per core (8 heads x 64)
HL = 8            # local heads
NP = 4            # local head pairs
KT = D // 128     # 8 contraction tiles
NCORES = 8

_CACHE = {}
LAST_RESULT = None  # BassKernelResults of the most recent run (for profiling)


_DEBUG = os.environ.get("KBG_DEBUG") == "1"


def _build_program():
    # Bacc (not raw Bass): its compile pipeline splits multi-sem waits into
    # EventSemaphore instructions and moves matmul waits onto ldweights --
    # walrus rejects >1 sync wait on most instructions.
    nc = bacc.Bacc()
    dbg = {}
    if _DEBUG:
        dbg["qt"] = nc.declare_dram_parameter("dbg_qt", [128, S], F32, isOutput=True)
        dbg["kt"] = nc.declare_dram_parameter("dbg_kt", [128, S], F32, isOutput=True)
        dbg["va"] = nc.declare_dram_parameter("dbg_va", [128, 520], F32, isOutput=True)
        dbg["st"] = nc.declare_dram_parameter("dbg_st", [128, 1024], F32, isOutput=True)
        dbg["pt"] = nc.declare_dram_parameter("dbg_pt", [128, 1024], F32, isOutput=True)
        dbg["ctx"] = nc.declare_dram_parameter("dbg_ctx", [65, 512], F32, isOutput=True)
        dbg["dn"] = nc.declare_dram_parameter("dbg_dn", [1, 1024], F32, isOutput=True)
        dbg["rc"] = nc.declare_dram_parameter("dbg_rc", [1, 1024], F32, isOutput=True)
        dbg["bcs"] = nc.declare_dram_parameter("dbg_bcs", [64, 512], F32, isOutput=True)
    xT_h = nc.declare_dram_parameter("xT", [D, S], BF16, isOutput=False)
    eT_h = nc.declare_dram_parameter("eT", [D, S], BF16, isOutput=False)
    wqT_h = nc.declare_dram_parameter("wqT", [D, DL], BF16, isOutput=False)
    wkT_h = nc.declare_dram_parameter("wkT", [D, DL], BF16, isOutput=False)
    wvT_h = nc.declare_dram_parameter("wvT", [D, DL], BF16, isOutput=False)
    outT_h = nc.declare_dram_parameter("outT", [DL, S], F32, isOutput=True)

    # [D, N] viewed as [128, KT, N]: partition p, ktile k -> row k*128+p
    xT_v = xT_h[:].rearrange("(k p) n -> p k n", p=128)
    eT_v = eT_h[:].rearrange("(k p) n -> p k n", p=128)
    wqT_v = wqT_h[:].rearrange("(k p) n -> p k n", p=128)
    wkT_v = wkT_h[:].rearrange("(k p) n -> p k n", p=128)
    wvT_v = wvT_h[:].rearrange("(k p) n -> p k n", p=128)

    with tile.TileContext(nc) as tc:
        with tc.tile_pool(name="persist", bufs=1) as persist:
            qT = persist.tile([128, NP, S], BF16, tag="qT")
            kT = persist.tile([128, NP, S], BF16, tag="kT")
            # v augmented: per SK tile, per head: 64 v-dims + ones column
            vA = persist.tile([128, 16, HL, 65], BF16, tag="vA")
            zbias = persist.tile([128, 1], F32, tag="zbias")

            nc.vector.memset(zbias[:], 0.0)
            nc.vector.memset(vA[:, :, :, 64:65], 1.0)

            # ---------------- phase 1: projections ----------------
            with (
                tc.tile_pool(name="ph1", bufs=2) as ph1,
                tc.tile_pool(name="wp", bufs=1) as wp,
                tc.tile_pool(name="ps1", bufs=3, space=PSUM) as ps1,
            ):
                wq = wp.tile([128, KT, DL], BF16, tag="wq")
                wk = wp.tile([128, KT, DL], BF16, tag="wk")
                wv = wp.tile([128, KT, DL], BF16, tag="wv")
                nc.sync.dma_start(wq[:], wqT_v)
                nc.sync.dma_start(wk[:], wkT_v)
                nc.sync.dma_start(wv[:], wvT_v)

                # kT and v from encoder states; each eT chunk read once
                for n in range(4):
                    nsl = slice(n * 512, (n + 1) * 512)
                    et = ph1.tile([128, KT, 512], BF16, tag="act")
                    nc.sync.dma_start(et[:], eT_v[:, :, nsl])
                    for m in range(4):
                        ps = ps1.tile([128, 512], F32, tag="pp")
                        msl = slice(m * 128, (m + 1) * 128)
                        for k in range(KT):
                            nc.tensor.matmul(
                                ps[:],
                                wk[:, k, msl],
                                et[:, k, :],
                                start=(k == 0),
                                stop=(k == KT - 1),
                            )
                        nc.vector.tensor_copy(kT[:, m, nsl], ps[:])
                    for sub in range(4):
                        mj = n * 4 + sub
                        ps = ps1.tile([128, 512], F32, tag="pp")
                        ssl = slice(sub * 128, (sub + 1) * 128)
                        for k in range(KT):
                            nc.tensor.matmul(
                                ps[:],
                                et[:, k, ssl],
                                wv[:, k, :],
                                start=(k == 0),
                                stop=(k == KT - 1),
                            )
                        nc.vector.tensor_copy(
                            vA[:, mj, :, 0:64],
                            ps[:].rearrange("p (h d) -> p h d", h=HL),
                        )

                # qT from hidden states
                for n in range(4):
                    nsl = slice(n * 512, (n + 1) * 512)
                    xt = ph1.tile([128, KT, 512], BF16, tag="act")
                    nc.sync.dma_start(xt[:], xT_v[:, :, nsl])
                    for m in range(4):
                        ps = ps1.tile([128, 512], F32, tag="pp")
                        msl = slice(m * 128, (m + 1) * 128)
                        for k in range(KT):
                            nc.tensor.matmul(
                                ps[:],
                                wq[:, k, msl],
                                xt[:, k, :],
                                start=(k == 0),
                                stop=(k == KT - 1),
                            )
                        nc.vector.tensor_copy(qT[:, m, nsl], ps[:])

            # ---------------- phase 2: attention ----------------
            with (
                tc.tile_pool(name="stp", bufs=2, space=PSUM) as stp,
                tc.tile_pool(name="ctxp", bufs=4, space=PSUM) as ctxp,
                tc.tile_pool(name="ptp", bufs=3) as ptp,
                tc.tile_pool(name="stg", bufs=2) as stgp,
                tc.tile_pool(name="dnp", bufs=2) as dnp,
                tc.tile_pool(name="dbgp", bufs=1) as dbgp,
            ):
                if _DEBUG:
                    t = dbgp.tile([128, S], F32, tag="dbg_qt")
                    nc.vector.tensor_copy(t[:], qT[:, 0, :])
                    nc.sync.dma_start(dbg["qt"][:], t[:])
                    t = dbgp.tile([128, S], F32, tag="dbg_kt")
                    nc.vector.tensor_copy(t[:], kT[:, 0, :])
                    nc.sync.dma_start(dbg["kt"][:], t[:])
                    t = dbgp.tile([128, 8, 65], F32, tag="dbg_va")
                    nc.vector.tensor_copy(t[:], vA[:, 0, :, :])
                    nc.sync.dma_start(
                        dbg["va"][:].rearrange("p (h d) -> p h d", h=HL), t[:]
                    )
                for p in range(NP):
                    stage_a = stgp.tile([64, S], F32, tag="stage_a")
                    stage_b = stgp.tile([64, S], F32, tag="stage_b")
                    for c in range(4):
                        csl = slice(c * 512, (c + 1) * 512)
                        ctx_a = ctxp.tile([65, 512], F32, tag="ctx")
                        ctx_b = ctxp.tile([65, 512], F32, tag="ctx")
                        for j in range(16):
                            jsl = slice(j * 128, (j + 1) * 128)
                            st = stp.tile([128, 1024], F32, tag="st")
                            nc.tensor.matmul(
                                st[:, 0:512],
                                kT[0:64, p, jsl],
                                qT[0:64, p, csl],
                                start=True,
                                stop=True,
                            )
                            nc.tensor.matmul(
                                st[:, 512:1024],
                                kT[64:128, p, jsl],
                                qT[64:128, p, csl],
                                start=True,
                                stop=True,
                            )
                            pt = ptp.tile([128, 1024], BF16, tag="pt")
                            nc.scalar.activation(pt[:], st[:], EXP, bias=zbias[:, 0:1])
                            nc.tensor.matmul(
                                ctx_a[:],
                                vA[:, j, 2 * p, :],
                                pt[:, 0:512],
                                start=(j == 0),
                                stop=(j == 15),
                            )
                            nc.tensor.matmul(
                                ctx_b[:],
                                vA[:, j, 2 * p + 1, :],
                                pt[:, 512:1024],
                                start=(j == 0),
                                stop=(j == 15),
                            )
                            if _DEBUG and p == 0 and c == 0 and j == 0:
                                t = dbgp.tile([128, 1024], F32, tag="dbg_st")
                                nc.vector.tensor_copy(t[:], st[:])
                                nc.sync.dma_start(dbg["st"][:], t[:])
                                t = dbgp.tile([128, 1024], F32, tag="dbg_pt")
                                nc.vector.tensor_copy(t[:], pt[:])
                                nc.sync.dma_start(dbg["pt"][:], t[:])
                        # denominators live in ctx row 64; keep everything at
                        # partition base 64 (32-aligned) until the broadcast
                        dn = dnp.tile([65, 1024], F32, tag="dn")
                        nc.vector.tensor_copy(dn[64:65, 0:512], ctx_a[64:65, :])
                        nc.vector.tensor_copy(dn[64:65, 512:1024], ctx_b[64:65, :])
                        # move the denom row to partition 0 (cross-partition
                        # needs a DMA), reciprocal there, then broadcast over
                        # 64 partitions on the idle GPSIMD engine -- the whole
                        # chain stays off the PE FIFO so the next chunk's
                        # matmuls never wait on the reciprocal
                        dn0 = dnp.tile([1, 1024], F32, tag="dn0")
                        nc.sync.dma_start(dn0[:], dn[64:65, :])
                        rc0 = dnp.tile([1, 1024], F32, tag="rc0")
                        nc.vector.reciprocal(rc0[:], dn0[:])
                        bcs_a = dnp.tile([64, 512], F32, tag="bcs_a")
                        bcs_b = dnp.tile([64, 512], F32, tag="bcs_b")
                        nc.gpsimd.partition_broadcast(bcs_a[:], rc0[0:1, 0:512])
                        nc.gpsimd.partition_broadcast(bcs_b[:], rc0[0:1, 512:1024])
                        if _DEBUG and p == 0 and c == 0:
                            t = dbgp.tile([65, 512], F32, tag="dbg_ctx")
                            nc.vector.tensor_copy(t[:], ctx_a[:])
                            nc.sync.dma_start(dbg["ctx"][:], t[:])
                            nc.sync.dma_start(dbg["dn"][:], dn[64:65, :])
                            nc.sync.dma_start(dbg["rc"][:], rc0[:])
                            nc.sync.dma_start(dbg["bcs"][:], bcs_a[:])
                        nc.vector.tensor_mul(
                            stage_a[:, csl], ctx_a[0:64, :], bcs_a[:]
                        )
                        nc.vector.tensor_mul(
                            stage_b[:, csl], ctx_b[0:64, :], bcs_b[:]
                        )
                    nc.sync.dma_start(
                        outT_h[p * 128 : p * 128 + 64, :], stage_a[:]
                    )
                    nc.sync.dma_start(
                        outT_h[p * 128 + 64 : (p + 1) * 128, :], stage_b[:]
                    )

    nc.finalize()
    return nc


def kernel(hidden_states, encoder_hidden_states, Wq, Wk, Wv):
    global LAST_RESULT
    hidden_states = np.asarray(hidden_states, dtype=np.float32)
    encoder_hidden_states = np.asarray(encoder_hidden_states, dtype=np.float32)
    Wq = np.asarray(Wq, dtype=np.float32)
    Wk = np.asarray(Wk, dtype=np.float32)
    Wv = np.asarray(Wv, dtype=np.float32)

    if "nc" not in _CACHE:
        _CACHE["nc"] = _build_program()
    nc = _CACHE["nc"]

    import ml_dtypes

    bf16 = ml_dtypes.bfloat16
    in_maps = []
    for c in range(NCORES):
        b, hg = divmod(c, 2)
        rsl = slice(hg * DL, (hg + 1) * DL)
        in_maps.append(
            {
                "xT": np.ascontiguousarray(hidden_states[b].T).astype(bf16),
                "eT": np.ascontiguousarray(encoder_hidden_states[b].T).astype(bf16),
                # fold the 1/sqrt(HD)=1/8 score scale into Wq
                "wqT": np.ascontiguousarray((Wq[rsl] * 0.125).T).astype(bf16),
                "wkT": np.ascontiguousarray(Wk[rsl].T).astype(bf16),
                "wvT": np.ascontiguousarray(Wv[rsl].T).astype(bf16),
            }
        )

    res = run_bass_kernel_spmd(nc, in_maps, list(range(NCORES)))
    LAST_RESULT = res

    out = np.empty((B, S, D), dtype=np.float32)
    for c in range(NCORES):
        b, hg = divmod(c, 2)
        out[b, :, hg * DL : (hg + 1) * DL] = res.results[c]["outT"].T
    return out
